# revision 16
# baseline (speedup 1.0000x reference)
"""BailingMoE linear attention (lightning attention) on 8 trn2 NeuronCores.

Tensor-parallel over heads: 2 heads per core. Full inputs in, full output out.
Per core: qkv+g projections (fp32r matmuls; q,k,g in [chan,seq] layout, v in
[seq,chan]), per-head RMSNorm + neox RoPE (half-swap via permutation matmul),
chunked linear attention with decayed kv state in SBUF, group RMSNorm + gate,
AllToAll to sequence-shard y, then the dense projection for the core's
1024-row output shard.
"""
import math

import numpy as np

S = 8192
HID = 2048
H = 16
D = 128
BLK = 256
GROUPS = 8
EPS = 1e-5
ROPE_THETA = 600000.0
SCALE = D ** -0.5
N_CORES = 8
HPC = H // N_CORES          # heads per core = 2
CPC = HPC * D               # channels per core = 256
KT = 17                     # contraction tiles (2048 hid + bias row, padded)
KPAD = KT * 128             # 2176
SEQ_G = 512                 # seq per projection group
NG = S // SEQ_G             # 16 groups
SB = S // N_CORES           # seq block per core after AllToAll = 1024
CPG = SEQ_G // BLK          # chunks per group = 2

_cache = {}


def _build_slopes():
    start = 2.0 ** (-(2.0 ** (-(math.log2(H) - 3.0))))
    slopes = np.array([start * start ** i for i in range(H)], dtype=np.float32)
    return slopes * np.float32(1.0 - 0.0 / (20 - 1) + 1e-5)


def _build_program():
    import concourse.bacc as bacc
    import concourse.tile as tile
    import concourse.mybir as mybir
    from contextlib import ExitStack

    dt = mybir.dt
    AF = mybir.ActivationFunctionType
    OP = mybir.AluOpType

    nc = bacc.Bacc("TRN2", target_bir_lowering=False, debug=False,
                   num_devices=N_CORES)

    def din(name, shape, dtype=dt.float32):
        return nc.dram_tensor(name, shape, dtype, kind="ExternalInput").ap()

    hsT = din("hsT", [KPAD, S], dt.float32r)
    wB = din("wB", [KPAD, 768], dt.float32r)       # cols: q(256) k(256) g(256)
    wv = din("wv", [KPAD, 256], dt.float32r)
    dwT = din("dwT", [HID, HID], dt.float32r)
    cosf = din("cosf", [128, S])                    # [cos; cos]
    sinf = din("sinf", [128, S])                    # [-sin; sin]
    qdec_d = din("qdec", [128, HPC, BLK])
    kdec_d = din("kdec", [128, HPC, BLK])
    diag_d = din("diagT", [128, HPC, 2, BLK])
    qnw_d = din("qnw", [128, 1])
    knw_d = din("knw", [128, 1])
    gnw_d = din("gnw", [128, HPC])
    blk_d = din("blkdec", [128, HPC])
    ones128_d = din("ones128", [128, 1], dt.float32r)
    onesr_d = din("onesr", [1, 128], dt.float32r)   # bcast lhsT, value 1
    scaler_d = din("scaler", [1, 128], dt.float32r)  # bcast lhsT, value SCALE
    idm_d = din("idm", [128, 128], dt.float32r)
    idsw_d = din("idsw", [128, 128], dt.float32r)   # half-swap permutation
    zkv_d = din("zkv", [128, 128], dt.float32r)     # zeros for kv init

    out_d = nc.dram_tensor("out", [SB, HID], dt.float32,
                           kind="ExternalOutput").ap()

    with tile.TileContext(nc) as tc:
        ctx = ExitStack()
        consts = ctx.enter_context(tc.tile_pool(name="consts", bufs=1))
        wpool = ctx.enter_context(tc.tile_pool(name="wpool", bufs=1))
        dramp = ctx.enter_context(tc.tile_pool(name="dramp", bufs=1, space="DRAM"))
        y_send = dramp.tile([N_CORES, CPC, SB], dt.float32r, name="y_send")
        y_recv = dramp.tile([N_CORES, CPC, SB], dt.float32r, name="y_recv")

        def cload(name, ap_src, shape, dtype=dt.float32):
            t = consts.tile(shape, dtype, name=name, tag=name)
            nc.sync.dma_start(out=t[:], in_=ap_src)
            return t

        qdec = cload("qdec_s", qdec_d[:], [128, HPC, BLK])
        kdec = cload("kdec_s", kdec_d[:], [128, HPC, BLK])
        diag = cload("diag_s", diag_d[:], [128, HPC, 2, BLK])
        qnw = cload("qnw_s", qnw_d[:], [128, 1])
        knw = cload("knw_s", knw_d[:], [128, 1])
        gnw = cload("gnw_s", gnw_d[:], [128, HPC])
        blkd = cload("blkd_s", blk_d[:], [128, HPC])
        ones128 = cload("ones128_s", ones128_d[:], [128, 1], dt.float32r)
        onesr = cload("onesr_s", onesr_d[:], [1, 128], dt.float32r)
        scaler = cload("scaler_s", scaler_d[:], [1, 128], dt.float32r)
        idm = cload("idm_s", idm_d[:], [128, 128], dt.float32r)
        idsw = cload("idsw_s", idsw_d[:], [128, 128], dt.float32r)
        epsb = consts.tile([1, 1], dt.float32, name="epsb", tag="epsb")
        nc.vector.memset(epsb[:], EPS)

        wB_sb = wpool.tile([128, KT, 768], dt.float32r, name="wB_sb")
        nc.sync.dma_start(
            out=wB_sb[:], in_=wB.rearrange("(t p) c -> p t c", p=128))
        wv_sb = wpool.tile([128, KT, 256], dt.float32r, name="wv_sb")
        nc.sync.dma_start(
            out=wv_sb[:], in_=wv.rearrange("(t p) c -> p t c", p=128))

        main = ExitStack()
        hkp = main.enter_context(tc.tile_pool(name="hkp", bufs=17))
        tabp = main.enter_context(tc.tile_pool(name="tabp", bufs=1))
        evp = main.enter_context(tc.tile_pool(name="evp", bufs=2))
        xrp = main.enter_context(tc.tile_pool(name="xrp", bufs=1))
        natp = main.enter_context(tc.tile_pool(name="natp", bufs=1))
        attp = main.enter_context(tc.tile_pool(name="attp", bufs=2))
        kvp = main.enter_context(tc.tile_pool(name="kvp", bufs=1))
        yp = main.enter_context(tc.tile_pool(name="yp", bufs=2))
        psp = main.enter_context(tc.tile_pool(name="psp", bufs=3, space="PSUM"))
        pse = main.enter_context(tc.tile_pool(name="pse", bufs=2, space="PSUM"))
        psa = main.enter_context(tc.tile_pool(name="psa", bufs=3, space="PSUM"))

        # persistent kv state, ping-pong per head
        kv_sb = [[kvp.tile([128, 128], dt.float32r, name=f"kv{h}_{i}",
                           tag=f"kv{h}_{i}") for i in range(2)]
                 for h in range(HPC)]
        for h in range(HPC):
            nc.sync.dma_start(out=kv_sb[h][0][:], in_=zkv_d[:])

        for g in range(NG):
            s0 = g * SEQ_G
            hk = []
            for t in range(KT):
                hkt = hkp.tile([128, SEQ_G], dt.float32r,
                               name=f"hk{g}_{t}", tag="hk")
                nc.sync.dma_start(out=hkt[:],
                                  in_=hsT[t * 128:(t + 1) * 128, s0:s0 + SEQ_G])
                hk.append(hkt)
            cos_g = tabp.tile([128, SEQ_G], dt.float32, name=f"cos{g}", tag="cos")
            nc.sync.dma_start(out=cos_g[:], in_=cosf[:, s0:s0 + SEQ_G])
            sin_g = tabp.tile([128, SEQ_G], dt.float32, name=f"sin{g}", tag="sin")
            nc.sync.dma_start(out=sin_g[:], in_=sinf[:, s0:s0 + SEQ_G])

            # ---- projection accumulations (all matmuls first) --------------
            accs = []
            for ci in range(6):  # 0,1=q  2,3=k  4,5=g
                acc = psp.tile([128, SEQ_G], dt.float32,
                               name=f"acc{g}_{ci}", tag="ps")
                for t in range(KT):
                    nc.tensor.matmul(acc[:], wB_sb[:, t, ci * 128:(ci + 1) * 128],
                                     hk[t][:], start=(t == 0), stop=(t == KT - 1))
                accs.append(acc)
                if ci < 4:
                    # free the accumulator early via ACT copy
                    xb = evp.tile([128, SEQ_G], dt.float32,
                                  name=f"xb{g}_{ci}", tag="xb", bufs=3)
                    nc.scalar.activation(xb[:], acc[:], AF.Copy)
                    accs[ci] = xb
                else:
                    # g gate: sigmoid via exp (frees acc)
                    eg = evp.tile([128, SEQ_G], dt.float32,
                                  name=f"eg{g}_{ci}", tag="eg")
                    nc.scalar.activation(eg[:], acc[:], AF.Exp, scale=-1.0)
                    accs[ci] = eg
            v_accs = []
            for s2 in range(2):
                accv = psp.tile([128, SEQ_G], dt.float32,
                                name=f"accv{g}_{s2}", tag="ps")
                for half in range(2):
                    st = s2 * 2 + half
                    for t in range(KT):
                        nc.tensor.matmul(
                            accv[:, half * 256:(half + 1) * 256],
                            hk[t][:, st * 128:(st + 1) * 128],
                            wv_sb[:, t, :],
                            start=(t == 0), stop=(t == KT - 1))
                v_accs.append(accv)

            # ---- v eviction (DVE, natural layout) --------------------------
            v_nat = []
            for s2 in range(2):
                for half in range(2):
                    st = s2 * 2 + half
                    vn = natp.tile([128, 256], dt.float32r,
                                   name=f"vn{g}_{st}", tag=f"vn{st}")
                    nc.vector.tensor_copy(
                        vn[:], v_accs[s2][:, half * 256:(half + 1) * 256])
                    v_nat.append(vn)

            # ---- q/k norm + rope, g gate -----------------------------------
            qr_t, kr_t, ktil_t, sig_t = [], [], [], []
            for ci in range(6):
                if ci < 4:
                    is_q = ci < 2
                    xb = accs[ci]
                    sq = evp.tile([128, SEQ_G], dt.float32r,
                                  name=f"sq{g}_{ci}", tag="sq")
                    nc.vector.tensor_tensor(out=sq[:], in0=xb[:], in1=xb[:],
                                            op=OP.mult)
                    ssq = evp.tile([1, SEQ_G], dt.float32,
                                    name=f"ssq{g}_{ci}", tag="ssq")
                    nc.gpsimd.tensor_reduce(ssq[:], sq[:],
                                            axis=mybir.AxisListType.C,
                                            op=OP.add)
                    lnt = evp.tile([1, SEQ_G], dt.float32,
                                   name=f"ln{g}_{ci}", tag="ln")
                    nc.scalar.activation(lnt[:], ssq[:], AF.Ln,
                                         bias=epsb[:], scale=1.0 / D)
                    rstd = evp.tile([1, SEQ_G], dt.float32r,
                                    name=f"rstd{g}_{ci}", tag="rstd")
                    nc.scalar.activation(rstd[:], lnt[:], AF.Exp, scale=-0.5)
                    bc = pse.tile([128, SEQ_G], dt.float32,
                                  name=f"bc{g}_{ci}", tag="pse")
                    nc.tensor.matmul(bc[:], scaler[:] if is_q else onesr[:],
                                     rstd[:], start=True, stop=True)
                    bcs = evp.tile([128, SEQ_G], dt.float32,
                                   name=f"bcs{g}_{ci}", tag="bcs")
                    nc.vector.tensor_copy(bcs[:], bc[:])
                    xn = evp.tile([128, SEQ_G], dt.float32r,
                                  name=f"xn{g}_{ci}", tag="xn")
                    nc.vector.scalar_tensor_tensor(
                        out=xn[:], in0=xb[:], scalar=qnw[:] if is_q else knw[:],
                        in1=bcs[:], op0=OP.mult, op1=OP.mult)
                    # rope: xr = xn*cos + swap(xn)*sin_signed
                    m1 = evp.tile([128, SEQ_G], dt.float32,
                                  name=f"m1{g}_{ci}", tag="m1", bufs=1)
                    nc.vector.tensor_tensor(out=m1[:], in0=xn[:], in1=cos_g[:],
                                            op=OP.mult)
                    swp = pse.tile([128, SEQ_G], dt.float32,
                                   name=f"swp{g}_{ci}", tag="pse")
                    nc.tensor.matmul(swp[:], idsw[:], xn[:],
                                     start=True, stop=True)
                    m2 = evp.tile([128, SEQ_G], dt.float32,
                                  name=f"m2{g}_{ci}", tag="m2", bufs=1)
                    nc.vector.tensor_tensor(out=m2[:], in0=swp[:], in1=sin_g[:],
                                            op=OP.mult)
                    xr = xrp.tile([128, SEQ_G], dt.float32r,
                                  name=f"xr{g}_{ci}", tag=f"xr{ci}")
                    nc.vector.tensor_tensor(out=xr[:], in0=m1[:], in1=m2[:],
                                            op=OP.add)
                    if is_q:
                        qr_t.append(xr)
                    else:
                        kr_t.append(xr)
                else:
                    eg = accs[ci]
                    nc.vector.tensor_scalar_add(eg[:], eg[:], 1.0)
                    sig = xrp.tile([128, SEQ_G], dt.float32,
                                   name=f"sig{g}_{ci}", tag=f"sig{ci}")
                    nc.vector.reciprocal(sig[:], eg[:])
                    sig_t.append(sig)

            # decayed k for the kv update, per chunk
            for h in range(HPC):
                ktil = xrp.tile([128, SEQ_G], dt.float32r,
                                name=f"ktil{g}_{h}", tag=f"ktil{h}")
                for cc in range(CPG):
                    nc.vector.tensor_tensor(
                        out=ktil[:, cc * BLK:(cc + 1) * BLK],
                        in0=kr_t[h][:, cc * BLK:(cc + 1) * BLK],
                        in1=kdec[:, h, :], op=OP.mult)
                ktil_t.append(ktil)

            # ---- k natural (transpose of decayed k) ------------------------
            knat = [[None] * CPG for _ in range(HPC)]
            for h in range(HPC):
                for cc in range(CPG):
                    kn_list = []
                    for j in range(2):
                        tp = pse.tile([128, 128], dt.float32r,
                                      name=f"tp{g}_{h}_{cc}_{j}", tag="pse")
                        nc.tensor.transpose(
                            tp[:],
                            ktil_t[h][:, cc * BLK + j * 128:cc * BLK + (j + 1) * 128],
                            idm[:])
                        kn = natp.tile([128, 128], dt.float32r,
                                       name=f"kn{g}_{h}_{cc}_{j}", tag="kn",
                                       bufs=8)
                        nc.vector.tensor_copy(kn[:], tp[:])
                        kn_list.append(kn)
                    knat[h][cc] = kn_list

            # ---- attention chunks ------------------------------------------
            for cc in range(CPG):
                ch = g * CPG + cc
                b = ch // (SB // BLK)
                off = (ch % (SB // BLK)) * BLK
                o_ps = []
                for h in range(HPC):
                    qr = qr_t[h][:, cc * BLK:(cc + 1) * BLK]
                    kv_cur = kv_sb[h][ch % 2]
                    kv_nxt = kv_sb[h][(ch + 1) % 2]
                    # kq[j, i] masked
                    kq = psa.tile([128, SEQ_G], dt.float32,
                                  name=f"kq{ch}_{h}", tag="psa")
                    kqd = []
                    for j in range(2):
                        nc.tensor.matmul(
                            kq[:, j * BLK:(j + 1) * BLK],
                            kr_t[h][:, cc * BLK + j * 128:cc * BLK + (j + 1) * 128],
                            qr, start=True, stop=True)
                        kqj = attp.tile([128, BLK], dt.float32r,
                                        name=f"kqd{ch}_{h}_{j}", tag="kqd",
                                        bufs=3)
                        nc.vector.tensor_tensor(
                            out=kqj[:], in0=kq[:, j * BLK:(j + 1) * BLK],
                            in1=diag[:, h, j, :], op=OP.mult)
                        kqd.append(kqj)
                    # q with decay
                    qt = attp.tile([128, BLK], dt.float32r,
                                   name=f"qt{ch}_{h}", tag="qt")
                    nc.vector.tensor_tensor(out=qt[:], in0=qr,
                                            in1=qdec[:, h, :], op=OP.mult)
                    # outT = v0.T@kqd0 + v1.T@kqd1 + kv.T@qt
                    ops = psa.tile([128, BLK], dt.float32,
                                   name=f"ops{ch}_{h}", tag="psa")
                    for j in range(2):
                        nc.tensor.matmul(
                            ops[:], v_nat[cc * 2 + j][:, h * 128:(h + 1) * 128],
                            kqd[j][:], start=(j == 0), stop=False)
                    nc.tensor.matmul(ops[:], kv_cur[:], qt[:],
                                     start=False, stop=True)
                    o_ps.append(ops)
                    # kv update
                    kvp_ps = psa.tile([128, 128], dt.float32,
                                      name=f"kvp{ch}_{h}", tag="psa")
                    for j in range(2):
                        nc.tensor.matmul(
                            kvp_ps[:], knat[h][cc][j][:],
                            v_nat[cc * 2 + j][:, h * 128:(h + 1) * 128],
                            start=(j == 0), stop=(j == 1))
                    nc.vector.scalar_tensor_tensor(
                        out=kv_nxt[:], in0=kv_cur[:], scalar=blkd[:, h:h + 1],
                        in1=kvp_ps[:], op0=OP.mult, op1=OP.add)

                # group rmsnorm over both heads + gate
                sqs = []
                for h in range(HPC):
                    sqh = attp.tile([128, BLK], dt.float32r,
                                    name=f"gsq{ch}_{h}", tag="gsq", bufs=2)
                    nc.scalar.activation(sqh[:], o_ps[h][:], AF.Square)
                    sqs.append(sqh)
                gss = []
                for h in range(HPC):
                    gsh = attp.tile([1, BLK], dt.float32,
                                    name=f"gss{ch}_{h}", tag="gss", bufs=3)
                    nc.gpsimd.tensor_reduce(gsh[:], sqs[h][:],
                                            axis=mybir.AxisListType.C,
                                            op=OP.add)
                    gss.append(gsh)
                gssq = attp.tile([1, BLK], dt.float32,
                                 name=f"gssq{ch}", tag="gssq")
                nc.vector.tensor_tensor(out=gssq[:], in0=gss[0][:],
                                        in1=gss[1][:], op=OP.add)
                glt = attp.tile([1, BLK], dt.float32, name=f"glt{ch}", tag="glt")
                nc.scalar.activation(glt[:], gssq[:], AF.Ln,
                                     bias=epsb[:], scale=1.0 / CPC)
                grstd = attp.tile([1, BLK], dt.float32r,
                                  name=f"grstd{ch}", tag="grstd")
                nc.scalar.activation(grstd[:], glt[:], AF.Exp, scale=-0.5)
                gbc = psa.tile([128, BLK], dt.float32, name=f"gbc{ch}", tag="psa")
                nc.tensor.matmul(gbc[:], onesr[:], grstd[:],
                                 start=True, stop=True)
                gbcs = attp.tile([128, BLK], dt.float32,
                                 name=f"gbcs{ch}", tag="gbcs")
                nc.vector.tensor_copy(gbcs[:], gbc[:])
                for h in range(HPC):
                    y1 = yp.tile([128, BLK], dt.float32,
                                 name=f"y1{ch}_{h}", tag="y1")
                    nc.vector.scalar_tensor_tensor(
                        out=y1[:], in0=o_ps[h][:], scalar=gnw[:, h:h + 1],
                        in1=gbcs[:], op0=OP.mult, op1=OP.mult)
                    y2 = yp.tile([128, BLK], dt.float32r,
                                 name=f"y2{ch}_{h}", tag="y2", bufs=3)
                    nc.vector.tensor_tensor(
                        out=y2[:], in0=y1[:],
                        in1=sig_t[h][:, cc * BLK:(cc + 1) * BLK], op=OP.mult)
                    nc.sync.dma_start(
                        out=y_send[b, h * 128:(h + 1) * 128, off:off + BLK],
                        in_=y2[:])

        main.close()

        # ---- all-to-all ----------------------------------------------------
        nc.gpsimd.collective_compute(
            "AllToAll", mybir.AluOpType.bypass,
            replica_groups=[list(range(N_CORES))],
            ins=[y_send.opt()],
            outs=[y_recv.opt()],
        )

        # ---- dense ---------------------------------------------------------
        dctx = ExitStack()
        dwp = dctx.enter_context(tc.tile_pool(name="dwp", bufs=16))
        dyp = dctx.enter_context(tc.tile_pool(name="dyp", bufs=6))
        dop = dctx.enter_context(tc.tile_pool(name="dop", bufs=3))
        dps = dctx.enter_context(tc.tile_pool(name="dps", bufs=8, space="PSUM"))
        yv = y_recv  # [8, 256, SB] == [2048 chan, SB]
        for hh in range(2):
            for sq2 in range(2):
                accs = [[dps.tile([128, 512], dt.float32,
                                  name=f"dacc{hh}_{sq2}_{st}_{h2}", tag="dps")
                         for h2 in range(2)] for st in range(4)]
                for ct in range(16):
                    dw = dwp.tile([128, 1024], dt.float32r,
                                  name=f"dw{hh}_{sq2}_{ct}", tag="dw")
                    nc.sync.dma_start(
                        out=dw[:, 0:512],
                        in_=dwT[ct * 128:(ct + 1) * 128,
                                hh * 1024:hh * 1024 + 512])
                    nc.sync.dma_start(
                        out=dw[:, 512:1024],
                        in_=dwT[ct * 128:(ct + 1) * 128,
                                hh * 1024 + 512:(hh + 1) * 1024])
                    yt = dyp.tile([128, 512], dt.float32r,
                                  name=f"yt{hh}_{sq2}_{ct}", tag="yt")
                    nc.sync.dma_start(
                        out=yt[:],
                        in_=yv[ct // 2, (ct % 2) * 128:(ct % 2) * 128 + 128,
                               sq2 * 512:(sq2 + 1) * 512])
                    for st in range(4):
                        for h2 in range(2):
                            nc.tensor.matmul(
                                accs[st][h2][:],
                                yt[:, st * 128:(st + 1) * 128],
                                dw[:, h2 * 512:(h2 + 1) * 512],
                                start=(ct == 0), stop=(ct == 15))
                for st in range(4):
                    for h2 in range(2):
                        ot = dop.tile([128, 512], dt.float32,
                                      name=f"ot{hh}_{sq2}_{st}_{h2}", tag="ot")
                        nc.scalar.activation(ot[:], accs[st][h2][:], AF.Copy)
                        srow = sq2 * 512 + st * 128
                        nc.sync.dma_start(
                            out=out_d[srow:srow + 128,
                                      hh * 1024 + h2 * 512:hh * 1024 + (h2 + 1) * 512],
                            in_=ot[:])
        dctx.close()
        ctx.close()

    nc.compile()
    return nc


def _stage(hidden_states, positions, qkv_w, qkv_b, q_norm_w, k_norm_w,
           g_w, g_norm_w, dense_w):
    f32 = np.float32
    hidden_states = np.asarray(hidden_states, dtype=f32)
    positions = np.asarray(positions)
    qkv_w = np.asarray(qkv_w, dtype=f32)
    qkv_b = np.asarray(qkv_b, dtype=f32)
    q_norm_w = np.asarray(q_norm_w, dtype=f32)
    k_norm_w = np.asarray(k_norm_w, dtype=f32)
    g_w = np.asarray(g_w, dtype=f32)
    g_norm_w = np.asarray(g_norm_w, dtype=f32)
    dense_w = np.asarray(dense_w, dtype=f32)
    slopes = _build_slopes()

    hsT = np.zeros((KPAD, S), dtype=f32)
    hsT[0:HID] = hidden_states.T
    hsT[HID] = 1.0

    inv_freq = 1.0 / (ROPE_THETA ** (np.arange(0, D, 2, dtype=f32) / D))
    freqs = positions.astype(f32)[:, None] * inv_freq[None, :]  # [S, 64]
    cos = np.cos(freqs).T.astype(f32)     # [64, S]
    sin = np.sin(freqs).T.astype(f32)
    cosf = np.ascontiguousarray(np.concatenate([cos, cos], axis=0))
    sinf = np.ascontiguousarray(np.concatenate([-sin, sin], axis=0))

    idx = np.arange(BLK, dtype=f32)
    dwT = np.ascontiguousarray(dense_w.T).astype(f32)
    ones128 = np.ones((128, 1), dtype=f32)
    onesr = np.ones((1, 128), dtype=f32)
    scaler = np.full((1, 128), SCALE, dtype=f32)
    idm = np.eye(128, dtype=f32)
    idsw = np.zeros((128, 128), dtype=f32)
    for m in range(128):
        idsw[(m + 64) % 128, m] = 1.0
    qnw = q_norm_w.reshape(128, 1).copy()
    knw = k_norm_w.reshape(128, 1).copy()

    in_maps = []
    for j in range(N_CORES):
        heads = [j * HPC + h for h in range(HPC)]
        c0 = j * CPC
        wBm = np.zeros((KPAD, 768), dtype=f32)
        wBm[0:HID, 0:256] = qkv_w[c0:c0 + CPC, :].T
        wBm[0:HID, 256:512] = qkv_w[HID + c0:HID + c0 + CPC, :].T
        wBm[0:HID, 512:768] = g_w[c0:c0 + CPC, :].T
        wBm[HID, 0:256] = qkv_b[c0:c0 + CPC]
        wBm[HID, 256:512] = qkv_b[HID + c0:HID + c0 + CPC]
        wvm = np.zeros((KPAD, 256), dtype=f32)
        wvm[0:HID] = qkv_w[2 * HID + c0:2 * HID + c0 + CPC, :].T
        wvm[HID] = qkv_b[2 * HID + c0:2 * HID + c0 + CPC]

        sl = slopes[heads]  # [HPC]
        qdec = np.exp(-sl[:, None] * (idx + 1.0)[None, :]).astype(f32)
        qdec = np.ascontiguousarray(
            np.broadcast_to(qdec[None, :, :], (128, HPC, BLK)))
        kd = np.exp(-sl[:, None] * (BLK - 1.0 - idx)[None, :]).astype(f32)
        kdecm = np.ascontiguousarray(
            np.broadcast_to(kd[None, :, :], (128, HPC, BLK)))
        dif = idx[:, None] - idx[None, :]           # [i, j]
        diagT = np.zeros((128, HPC, 2, BLK), dtype=f32)
        for hh in range(HPC):
            dd = np.where(
                dif >= 0,
                np.exp(-sl[hh] * np.where(dif >= 0, dif, 0.0)),
                0.0).astype(f32)                    # [i, j]
            ddT = dd.T                               # [j, i]
            diagT[:, hh, 0, :] = ddT[0:128]
            diagT[:, hh, 1, :] = ddT[128:256]
        blkdec = np.ascontiguousarray(np.broadcast_to(
            np.exp(-sl * BLK).astype(f32)[None, :], (128, HPC)))
        gnwm = np.ascontiguousarray(g_norm_w[c0:c0 + CPC].reshape(HPC, 128).T)

        in_maps.append({
            "hsT": hsT, "wB": wBm, "wv": wvm, "dwT": dwT,
            "cosf": cosf, "sinf": sinf,
            "qdec": qdec, "kdec": kdecm, "diagT": diagT,
            "qnw": qnw, "knw": knw, "gnw": gnwm, "blkdec": blkdec,
            "ones128": ones128, "onesr": onesr, "scaler": scaler,
            "idm": idm, "idsw": idsw, "zkv": np.zeros((128, 128), dtype=f32),
        })
    return in_maps


def kernel(**inputs):
    from concourse.bass_utils import run_bass_kernel_spmd

    if "nc" not in _cache:
        _cache["nc"] = _build_program()
    nc = _cache["nc"]
    in_maps = _stage(**inputs)
    res = run_bass_kernel_spmd(nc, in_maps, list(range(N_CORES)))
    out = np.concatenate([res.results[j]["out"] for j in range(N_CORES)],
                         axis=0)
    return out.astype(np.float32)


# revision 17
# speedup vs baseline: 5.3590x; 5.3590x over previous
"""BailingMoE linear attention (lightning attention) on 8 trn2 NeuronCores.

Tensor-parallel over heads: 2 heads per core. Full inputs in, full output out.
Per core: qkv+g projections (fp32r matmuls; q,k,g in [chan,seq] layout, v in
[seq,chan]), per-head RMSNorm + neox RoPE (half-swap via permutation matmul),
chunked linear attention with decayed kv state in SBUF, group RMSNorm + gate,
AllToAll to sequence-shard y, then the dense projection for the core's
1024-row output shard.
"""
import math

import numpy as np

S = 8192
HID = 2048
H = 16
D = 128
BLK = 256
GROUPS = 8
EPS = 1e-5
ROPE_THETA = 600000.0
SCALE = D ** -0.5
N_CORES = 8
HPC = H // N_CORES          # heads per core = 2
CPC = HPC * D               # channels per core = 256
KT = 17                     # contraction tiles (2048 hid + bias row, padded)
KPAD = KT * 128             # 2176
SEQ_G = 512                 # seq per projection group
NG = S // SEQ_G             # 16 groups
SB = S // N_CORES           # seq block per core after AllToAll = 1024
CPG = SEQ_G // BLK          # chunks per group = 2

_cache = {}


def _build_slopes():
    start = 2.0 ** (-(2.0 ** (-(math.log2(H) - 3.0))))
    slopes = np.array([start * start ** i for i in range(H)], dtype=np.float32)
    return slopes * np.float32(1.0 - 0.0 / (20 - 1) + 1e-5)


def _build_program():
    import concourse.bacc as bacc
    import concourse.tile as tile
    import concourse.mybir as mybir
    from contextlib import ExitStack

    dt = mybir.dt
    AF = mybir.ActivationFunctionType
    OP = mybir.AluOpType

    nc = bacc.Bacc("TRN2", target_bir_lowering=False, debug=False,
                   num_devices=N_CORES)

    def din(name, shape, dtype=dt.float32):
        return nc.dram_tensor(name, shape, dtype, kind="ExternalInput").ap()

    hsT = din("hsT", [KPAD, S], dt.float32r)
    wB = din("wB", [KPAD, 768], dt.float32r)       # cols: q(256) k(256) g(256)
    wv = din("wv", [KPAD, 256], dt.float32r)
    dwT = din("dwT", [HID, HID], dt.float32r)
    cosf = din("cosf", [128, S])                    # [cos; cos]
    sinf = din("sinf", [128, S])                    # [-sin; sin]
    qdec_d = din("qdec", [128, HPC, BLK])
    kdec_d = din("kdec", [128, HPC, BLK])
    diag_d = din("diagT", [128, HPC, 2, BLK])
    qnw_d = din("qnw", [128, 1])
    knw_d = din("knw", [128, 1])
    gnw_d = din("gnw", [128, HPC])
    blk_d = din("blkdec", [128, HPC])
    ones128_d = din("ones128", [128, 1], dt.float32r)
    onesr_d = din("onesr", [1, 128], dt.float32r)   # bcast lhsT, value 1
    scaler_d = din("scaler", [1, 128], dt.float32r)  # bcast lhsT, value SCALE
    idm_d = din("idm", [128, 128], dt.float32r)
    idsw_d = din("idsw", [128, 128], dt.float32r)   # half-swap permutation
    zkv_d = din("zkv", [128, 128], dt.float32r)     # zeros for kv init

    out_d = nc.dram_tensor("out", [SB, HID], dt.float32,
                           kind="ExternalOutput").ap()

    with tile.TileContext(nc) as tc:
        ctx = ExitStack()
        consts = ctx.enter_context(tc.tile_pool(name="consts", bufs=1))
        wpool = ctx.enter_context(tc.tile_pool(name="wpool", bufs=1))
        dramp = ctx.enter_context(tc.tile_pool(name="dramp", bufs=1, space="DRAM"))
        y_send = dramp.tile([N_CORES, CPC, SB], dt.float32r, name="y_send")
        y_recv = dramp.tile([N_CORES, CPC, SB], dt.float32r, name="y_recv")

        def cload(name, ap_src, shape, dtype=dt.float32):
            t = consts.tile(shape, dtype, name=name, tag=name)
            nc.sync.dma_start(out=t[:], in_=ap_src)
            return t

        qdec = cload("qdec_s", qdec_d[:], [128, HPC, BLK])
        kdec = cload("kdec_s", kdec_d[:], [128, HPC, BLK])
        diag = cload("diag_s", diag_d[:], [128, HPC, 2, BLK])
        qnw = cload("qnw_s", qnw_d[:], [128, 1])
        knw = cload("knw_s", knw_d[:], [128, 1])
        gnw = cload("gnw_s", gnw_d[:], [128, HPC])
        blkd = cload("blkd_s", blk_d[:], [128, HPC])
        ones128 = cload("ones128_s", ones128_d[:], [128, 1], dt.float32r)
        onesr = cload("onesr_s", onesr_d[:], [1, 128], dt.float32r)
        scaler = cload("scaler_s", scaler_d[:], [1, 128], dt.float32r)
        idm = cload("idm_s", idm_d[:], [128, 128], dt.float32r)
        idsw = cload("idsw_s", idsw_d[:], [128, 128], dt.float32r)
        epsb = consts.tile([1, 1], dt.float32, name="epsb", tag="epsb")
        nc.vector.memset(epsb[:], EPS)

        wB_sb = wpool.tile([128, KT, 768], dt.float32r, name="wB_sb")
        nc.sync.dma_start(
            out=wB_sb[:], in_=wB.rearrange("(t p) c -> p t c", p=128))
        wv_sb = wpool.tile([128, KT, 256], dt.float32r, name="wv_sb")
        nc.sync.dma_start(
            out=wv_sb[:], in_=wv.rearrange("(t p) c -> p t c", p=128))

        main = ExitStack()
        hkp = main.enter_context(tc.tile_pool(name="hkp", bufs=17))
        tabp = main.enter_context(tc.tile_pool(name="tabp", bufs=1))
        evp = main.enter_context(tc.tile_pool(name="evp", bufs=2))
        xrp = main.enter_context(tc.tile_pool(name="xrp", bufs=1))
        natp = main.enter_context(tc.tile_pool(name="natp", bufs=1))
        attp = main.enter_context(tc.tile_pool(name="attp", bufs=2))
        kvp = main.enter_context(tc.tile_pool(name="kvp", bufs=1))
        yp = main.enter_context(tc.tile_pool(name="yp", bufs=2))
        psp = main.enter_context(tc.tile_pool(name="psp", bufs=3, space="PSUM"))
        pse = main.enter_context(tc.tile_pool(name="pse", bufs=2, space="PSUM"))
        psa = main.enter_context(tc.tile_pool(name="psa", bufs=3, space="PSUM"))

        # persistent kv state, ping-pong per head
        kv_sb = [[kvp.tile([128, 128], dt.float32r, name=f"kv{h}_{i}",
                           tag=f"kv{h}_{i}") for i in range(2)]
                 for h in range(HPC)]
        for h in range(HPC):
            nc.sync.dma_start(out=kv_sb[h][0][:], in_=zkv_d[:])

        for g in range(NG):
            s0 = g * SEQ_G
            hk = []
            for t in range(KT):
                hkt = hkp.tile([128, SEQ_G], dt.float32r,
                               name=f"hk{g}_{t}", tag="hk")
                nc.sync.dma_start(out=hkt[:],
                                  in_=hsT[t * 128:(t + 1) * 128, s0:s0 + SEQ_G])
                hk.append(hkt)
            cos_g = tabp.tile([128, SEQ_G], dt.float32, name=f"cos{g}", tag="cos")
            nc.sync.dma_start(out=cos_g[:], in_=cosf[:, s0:s0 + SEQ_G])
            sin_g = tabp.tile([128, SEQ_G], dt.float32, name=f"sin{g}", tag="sin")
            nc.sync.dma_start(out=sin_g[:], in_=sinf[:, s0:s0 + SEQ_G])

            # ---- projection accumulations (all matmuls first) --------------
            accs = []
            for ci in range(6):  # 0,1=q  2,3=k  4,5=g
                acc = psp.tile([128, SEQ_G], dt.float32,
                               name=f"acc{g}_{ci}", tag="ps")
                for t in range(KT):
                    nc.tensor.matmul(acc[:], wB_sb[:, t, ci * 128:(ci + 1) * 128],
                                     hk[t][:], start=(t == 0), stop=(t == KT - 1))
                accs.append(acc)
                if ci < 4:
                    # free the accumulator early via ACT copy
                    xb = evp.tile([128, SEQ_G], dt.float32,
                                  name=f"xb{g}_{ci}", tag="xb", bufs=3)
                    nc.scalar.activation(xb[:], acc[:], AF.Copy)
                    accs[ci] = xb
                else:
                    # g gate: sigmoid via exp (frees acc)
                    eg = evp.tile([128, SEQ_G], dt.float32,
                                  name=f"eg{g}_{ci}", tag="eg")
                    nc.scalar.activation(eg[:], acc[:], AF.Exp, scale=-1.0)
                    accs[ci] = eg
            v_accs = []
            for s2 in range(2):
                accv = psp.tile([128, SEQ_G], dt.float32,
                                name=f"accv{g}_{s2}", tag="ps")
                for half in range(2):
                    st = s2 * 2 + half
                    for t in range(KT):
                        nc.tensor.matmul(
                            accv[:, half * 256:(half + 1) * 256],
                            hk[t][:, st * 128:(st + 1) * 128],
                            wv_sb[:, t, :],
                            start=(t == 0), stop=(t == KT - 1))
                v_accs.append(accv)

            # ---- v eviction (DVE, natural layout) --------------------------
            v_nat = []
            for s2 in range(2):
                for half in range(2):
                    st = s2 * 2 + half
                    vn = natp.tile([128, 256], dt.float32r,
                                   name=f"vn{g}_{st}", tag=f"vn{st}")
                    nc.vector.tensor_copy(
                        vn[:], v_accs[s2][:, half * 256:(half + 1) * 256])
                    v_nat.append(vn)

            # ---- q/k norm + rope, g gate -----------------------------------
            qr_t, kr_t, ktil_t, sig_t = [], [], [], []
            for ci in range(6):
                if ci < 4:
                    is_q = ci < 2
                    xb = accs[ci]
                    sq = evp.tile([128, SEQ_G], dt.float32r,
                                  name=f"sq{g}_{ci}", tag="sq")
                    nc.vector.tensor_tensor(out=sq[:], in0=xb[:], in1=xb[:],
                                            op=OP.mult)
                    ssq = pse.tile([1, SEQ_G], dt.float32,
                                   name=f"ssq{g}_{ci}", tag="pse")
                    nc.tensor.matmul(ssq[:], ones128[:], sq[:],
                                     start=True, stop=True)
                    lnt = evp.tile([1, SEQ_G], dt.float32,
                                   name=f"ln{g}_{ci}", tag="ln")
                    nc.scalar.activation(lnt[:], ssq[:], AF.Ln,
                                         bias=epsb[:], scale=1.0 / D)
                    rstd = evp.tile([1, SEQ_G], dt.float32r,
                                    name=f"rstd{g}_{ci}", tag="rstd")
                    nc.scalar.activation(rstd[:], lnt[:], AF.Exp, scale=-0.5)
                    bc = pse.tile([128, SEQ_G], dt.float32,
                                  name=f"bc{g}_{ci}", tag="pse")
                    nc.tensor.matmul(bc[:], scaler[:] if is_q else onesr[:],
                                     rstd[:], start=True, stop=True)
                    bcs = evp.tile([128, SEQ_G], dt.float32,
                                   name=f"bcs{g}_{ci}", tag="bcs")
                    nc.vector.tensor_copy(bcs[:], bc[:])
                    xn = evp.tile([128, SEQ_G], dt.float32r,
                                  name=f"xn{g}_{ci}", tag="xn")
                    nc.vector.scalar_tensor_tensor(
                        out=xn[:], in0=xb[:], scalar=qnw[:] if is_q else knw[:],
                        in1=bcs[:], op0=OP.mult, op1=OP.mult)
                    # rope: xr = xn*cos + swap(xn)*sin_signed
                    m1 = evp.tile([128, SEQ_G], dt.float32,
                                  name=f"m1{g}_{ci}", tag="m1", bufs=1)
                    nc.vector.tensor_tensor(out=m1[:], in0=xn[:], in1=cos_g[:],
                                            op=OP.mult)
                    swp = pse.tile([128, SEQ_G], dt.float32,
                                   name=f"swp{g}_{ci}", tag="pse")
                    nc.tensor.matmul(swp[:], idsw[:], xn[:],
                                     start=True, stop=True)
                    m2 = evp.tile([128, SEQ_G], dt.float32,
                                  name=f"m2{g}_{ci}", tag="m2", bufs=1)
                    nc.vector.tensor_tensor(out=m2[:], in0=swp[:], in1=sin_g[:],
                                            op=OP.mult)
                    xr = xrp.tile([128, SEQ_G], dt.float32r,
                                  name=f"xr{g}_{ci}", tag=f"xr{ci}")
                    nc.vector.tensor_tensor(out=xr[:], in0=m1[:], in1=m2[:],
                                            op=OP.add)
                    if is_q:
                        qr_t.append(xr)
                    else:
                        kr_t.append(xr)
                else:
                    eg = accs[ci]
                    nc.vector.tensor_scalar_add(eg[:], eg[:], 1.0)
                    sig = xrp.tile([128, SEQ_G], dt.float32,
                                   name=f"sig{g}_{ci}", tag=f"sig{ci}")
                    nc.vector.reciprocal(sig[:], eg[:])
                    sig_t.append(sig)

            # decayed k for the kv update, per chunk
            for h in range(HPC):
                ktil = xrp.tile([128, SEQ_G], dt.float32r,
                                name=f"ktil{g}_{h}", tag=f"ktil{h}")
                for cc in range(CPG):
                    nc.vector.tensor_tensor(
                        out=ktil[:, cc * BLK:(cc + 1) * BLK],
                        in0=kr_t[h][:, cc * BLK:(cc + 1) * BLK],
                        in1=kdec[:, h, :], op=OP.mult)
                ktil_t.append(ktil)

            # ---- k natural (transpose of decayed k) ------------------------
            knat = [[None] * CPG for _ in range(HPC)]
            for h in range(HPC):
                for cc in range(CPG):
                    kn_list = []
                    for j in range(2):
                        tp = pse.tile([128, 128], dt.float32r,
                                      name=f"tp{g}_{h}_{cc}_{j}", tag="pse")
                        nc.tensor.transpose(
                            tp[:],
                            ktil_t[h][:, cc * BLK + j * 128:cc * BLK + (j + 1) * 128],
                            idm[:])
                        kn = natp.tile([128, 128], dt.float32r,
                                       name=f"kn{g}_{h}_{cc}_{j}", tag="kn",
                                       bufs=8)
                        nc.vector.tensor_copy(kn[:], tp[:])
                        kn_list.append(kn)
                    knat[h][cc] = kn_list

            # ---- attention chunks ------------------------------------------
            for cc in range(CPG):
                ch = g * CPG + cc
                b = ch // (SB // BLK)
                off = (ch % (SB // BLK)) * BLK
                o_ps = []
                for h in range(HPC):
                    qr = qr_t[h][:, cc * BLK:(cc + 1) * BLK]
                    kv_cur = kv_sb[h][ch % 2]
                    kv_nxt = kv_sb[h][(ch + 1) % 2]
                    # kq[j, i] masked
                    kq = psa.tile([128, SEQ_G], dt.float32,
                                  name=f"kq{ch}_{h}", tag="psa")
                    kqd = []
                    for j in range(2):
                        nc.tensor.matmul(
                            kq[:, j * BLK:(j + 1) * BLK],
                            kr_t[h][:, cc * BLK + j * 128:cc * BLK + (j + 1) * 128],
                            qr, start=True, stop=True)
                        kqj = attp.tile([128, BLK], dt.float32r,
                                        name=f"kqd{ch}_{h}_{j}", tag="kqd",
                                        bufs=3)
                        nc.vector.tensor_tensor(
                            out=kqj[:], in0=kq[:, j * BLK:(j + 1) * BLK],
                            in1=diag[:, h, j, :], op=OP.mult)
                        kqd.append(kqj)
                    # q with decay
                    qt = attp.tile([128, BLK], dt.float32r,
                                   name=f"qt{ch}_{h}", tag="qt")
                    nc.vector.tensor_tensor(out=qt[:], in0=qr,
                                            in1=qdec[:, h, :], op=OP.mult)
                    # outT = v0.T@kqd0 + v1.T@kqd1 + kv.T@qt
                    ops = psa.tile([128, BLK], dt.float32,
                                   name=f"ops{ch}_{h}", tag="psa")
                    for j in range(2):
                        nc.tensor.matmul(
                            ops[:], v_nat[cc * 2 + j][:, h * 128:(h + 1) * 128],
                            kqd[j][:], start=(j == 0), stop=False)
                    nc.tensor.matmul(ops[:], kv_cur[:], qt[:],
                                     start=False, stop=True)
                    o_ps.append(ops)
                    # kv update
                    kvp_ps = psa.tile([128, 128], dt.float32,
                                      name=f"kvp{ch}_{h}", tag="psa")
                    for j in range(2):
                        nc.tensor.matmul(
                            kvp_ps[:], knat[h][cc][j][:],
                            v_nat[cc * 2 + j][:, h * 128:(h + 1) * 128],
                            start=(j == 0), stop=(j == 1))
                    nc.vector.scalar_tensor_tensor(
                        out=kv_nxt[:], in0=kv_cur[:], scalar=blkd[:, h:h + 1],
                        in1=kvp_ps[:], op0=OP.mult, op1=OP.add)

                # group rmsnorm over both heads + gate
                sqs = []
                for h in range(HPC):
                    sqh = attp.tile([128, BLK], dt.float32r,
                                    name=f"gsq{ch}_{h}", tag="gsq", bufs=2)
                    nc.scalar.activation(sqh[:], o_ps[h][:], AF.Square)
                    sqs.append(sqh)
                gssq = psa.tile([1, BLK], dt.float32, name=f"gssq{ch}", tag="psa")
                for h in range(HPC):
                    nc.tensor.matmul(gssq[:], ones128[:], sqs[h][:],
                                     start=(h == 0), stop=(h == HPC - 1))
                glt = attp.tile([1, BLK], dt.float32, name=f"glt{ch}", tag="glt")
                nc.scalar.activation(glt[:], gssq[:], AF.Ln,
                                     bias=epsb[:], scale=1.0 / CPC)
                grstd = attp.tile([1, BLK], dt.float32r,
                                  name=f"grstd{ch}", tag="grstd")
                nc.scalar.activation(grstd[:], glt[:], AF.Exp, scale=-0.5)
                gbc = psa.tile([128, BLK], dt.float32, name=f"gbc{ch}", tag="psa")
                nc.tensor.matmul(gbc[:], onesr[:], grstd[:],
                                 start=True, stop=True)
                gbcs = attp.tile([128, BLK], dt.float32,
                                 name=f"gbcs{ch}", tag="gbcs")
                nc.vector.tensor_copy(gbcs[:], gbc[:])
                for h in range(HPC):
                    y1 = yp.tile([128, BLK], dt.float32,
                                 name=f"y1{ch}_{h}", tag="y1")
                    nc.vector.scalar_tensor_tensor(
                        out=y1[:], in0=o_ps[h][:], scalar=gnw[:, h:h + 1],
                        in1=gbcs[:], op0=OP.mult, op1=OP.mult)
                    y2 = yp.tile([128, BLK], dt.float32r,
                                 name=f"y2{ch}_{h}", tag="y2", bufs=3)
                    nc.vector.tensor_tensor(
                        out=y2[:], in0=y1[:],
                        in1=sig_t[h][:, cc * BLK:(cc + 1) * BLK], op=OP.mult)
                    nc.sync.dma_start(
                        out=y_send[b, h * 128:(h + 1) * 128, off:off + BLK],
                        in_=y2[:])

        main.close()

        # ---- all-to-all ----------------------------------------------------
        nc.gpsimd.collective_compute(
            "AllToAll", mybir.AluOpType.bypass,
            replica_groups=[list(range(N_CORES))],
            ins=[y_send.opt()],
            outs=[y_recv.opt()],
        )

        # ---- dense ---------------------------------------------------------
        dctx = ExitStack()
        dwp = dctx.enter_context(tc.tile_pool(name="dwp", bufs=16))
        dyp = dctx.enter_context(tc.tile_pool(name="dyp", bufs=6))
        dop = dctx.enter_context(tc.tile_pool(name="dop", bufs=3))
        dps = dctx.enter_context(tc.tile_pool(name="dps", bufs=8, space="PSUM"))
        yv = y_recv  # [8, 256, SB] == [2048 chan, SB]
        for hh in range(2):
            for sq2 in range(2):
                accs = [[dps.tile([128, 512], dt.float32,
                                  name=f"dacc{hh}_{sq2}_{st}_{h2}", tag="dps")
                         for h2 in range(2)] for st in range(4)]
                for ct in range(16):
                    dw = dwp.tile([128, 1024], dt.float32r,
                                  name=f"dw{hh}_{sq2}_{ct}", tag="dw")
                    nc.sync.dma_start(
                        out=dw[:, 0:512],
                        in_=dwT[ct * 128:(ct + 1) * 128,
                                hh * 1024:hh * 1024 + 512])
                    nc.sync.dma_start(
                        out=dw[:, 512:1024],
                        in_=dwT[ct * 128:(ct + 1) * 128,
                                hh * 1024 + 512:(hh + 1) * 1024])
                    yt = dyp.tile([128, 512], dt.float32r,
                                  name=f"yt{hh}_{sq2}_{ct}", tag="yt")
                    nc.sync.dma_start(
                        out=yt[:],
                        in_=yv[ct // 2, (ct % 2) * 128:(ct % 2) * 128 + 128,
                               sq2 * 512:(sq2 + 1) * 512])
                    for st in range(4):
                        for h2 in range(2):
                            nc.tensor.matmul(
                                accs[st][h2][:],
                                yt[:, st * 128:(st + 1) * 128],
                                dw[:, h2 * 512:(h2 + 1) * 512],
                                start=(ct == 0), stop=(ct == 15))
                for st in range(4):
                    for h2 in range(2):
                        ot = dop.tile([128, 512], dt.float32,
                                      name=f"ot{hh}_{sq2}_{st}_{h2}", tag="ot")
                        nc.scalar.activation(ot[:], accs[st][h2][:], AF.Copy)
                        srow = sq2 * 512 + st * 128
                        nc.sync.dma_start(
                            out=out_d[srow:srow + 128,
                                      hh * 1024 + h2 * 512:hh * 1024 + (h2 + 1) * 512],
                            in_=ot[:])
        dctx.close()
        ctx.close()

    nc.compile()
    return nc


def _stage(hidden_states, positions, qkv_w, qkv_b, q_norm_w, k_norm_w,
           g_w, g_norm_w, dense_w):
    f32 = np.float32
    hidden_states = np.asarray(hidden_states, dtype=f32)
    positions = np.asarray(positions)
    qkv_w = np.asarray(qkv_w, dtype=f32)
    qkv_b = np.asarray(qkv_b, dtype=f32)
    q_norm_w = np.asarray(q_norm_w, dtype=f32)
    k_norm_w = np.asarray(k_norm_w, dtype=f32)
    g_w = np.asarray(g_w, dtype=f32)
    g_norm_w = np.asarray(g_norm_w, dtype=f32)
    dense_w = np.asarray(dense_w, dtype=f32)
    slopes = _build_slopes()

    hsT = np.zeros((KPAD, S), dtype=f32)
    hsT[0:HID] = hidden_states.T
    hsT[HID] = 1.0

    inv_freq = 1.0 / (ROPE_THETA ** (np.arange(0, D, 2, dtype=f32) / D))
    freqs = positions.astype(f32)[:, None] * inv_freq[None, :]  # [S, 64]
    cos = np.cos(freqs).T.astype(f32)     # [64, S]
    sin = np.sin(freqs).T.astype(f32)
    cosf = np.ascontiguousarray(np.concatenate([cos, cos], axis=0))
    sinf = np.ascontiguousarray(np.concatenate([-sin, sin], axis=0))

    idx = np.arange(BLK, dtype=f32)
    dwT = np.ascontiguousarray(dense_w.T).astype(f32)
    ones128 = np.ones((128, 1), dtype=f32)
    onesr = np.ones((1, 128), dtype=f32)
    scaler = np.full((1, 128), SCALE, dtype=f32)
    idm = np.eye(128, dtype=f32)
    idsw = np.zeros((128, 128), dtype=f32)
    for m in range(128):
        idsw[(m + 64) % 128, m] = 1.0
    qnw = q_norm_w.reshape(128, 1).copy()
    knw = k_norm_w.reshape(128, 1).copy()

    in_maps = []
    for j in range(N_CORES):
        heads = [j * HPC + h for h in range(HPC)]
        c0 = j * CPC
        wBm = np.zeros((KPAD, 768), dtype=f32)
        wBm[0:HID, 0:256] = qkv_w[c0:c0 + CPC, :].T
        wBm[0:HID, 256:512] = qkv_w[HID + c0:HID + c0 + CPC, :].T
        wBm[0:HID, 512:768] = g_w[c0:c0 + CPC, :].T
        wBm[HID, 0:256] = qkv_b[c0:c0 + CPC]
        wBm[HID, 256:512] = qkv_b[HID + c0:HID + c0 + CPC]
        wvm = np.zeros((KPAD, 256), dtype=f32)
        wvm[0:HID] = qkv_w[2 * HID + c0:2 * HID + c0 + CPC, :].T
        wvm[HID] = qkv_b[2 * HID + c0:2 * HID + c0 + CPC]

        sl = slopes[heads]  # [HPC]
        qdec = np.exp(-sl[:, None] * (idx + 1.0)[None, :]).astype(f32)
        qdec = np.ascontiguousarray(
            np.broadcast_to(qdec[None, :, :], (128, HPC, BLK)))
        kd = np.exp(-sl[:, None] * (BLK - 1.0 - idx)[None, :]).astype(f32)
        kdecm = np.ascontiguousarray(
            np.broadcast_to(kd[None, :, :], (128, HPC, BLK)))
        dif = idx[:, None] - idx[None, :]           # [i, j]
        diagT = np.zeros((128, HPC, 2, BLK), dtype=f32)
        for hh in range(HPC):
            dd = np.where(
                dif >= 0,
                np.exp(-sl[hh] * np.where(dif >= 0, dif, 0.0)),
                0.0).astype(f32)                    # [i, j]
            ddT = dd.T                               # [j, i]
            diagT[:, hh, 0, :] = ddT[0:128]
            diagT[:, hh, 1, :] = ddT[128:256]
        blkdec = np.ascontiguousarray(np.broadcast_to(
            np.exp(-sl * BLK).astype(f32)[None, :], (128, HPC)))
        gnwm = np.ascontiguousarray(g_norm_w[c0:c0 + CPC].reshape(HPC, 128).T)

        in_maps.append({
            "hsT": hsT, "wB": wBm, "wv": wvm, "dwT": dwT,
            "cosf": cosf, "sinf": sinf,
            "qdec": qdec, "kdec": kdecm, "diagT": diagT,
            "qnw": qnw, "knw": knw, "gnw": gnwm, "blkdec": blkdec,
            "ones128": ones128, "onesr": onesr, "scaler": scaler,
            "idm": idm, "idsw": idsw, "zkv": np.zeros((128, 128), dtype=f32),
        })
    return in_maps


def kernel(**inputs):
    from concourse.bass_utils import run_bass_kernel_spmd

    if "nc" not in _cache:
        _cache["nc"] = _build_program()
    nc = _cache["nc"]
    in_maps = _stage(**inputs)
    res = run_bass_kernel_spmd(nc, in_maps, list(range(N_CORES)))
    out = np.concatenate([res.results[j]["out"] for j in range(N_CORES)],
                         axis=0)
    return out.astype(np.float32)


# revision 20
# speedup vs baseline: 5.6810x; 1.0601x over previous
"""BailingMoE linear attention (lightning attention) on 8 trn2 NeuronCores.

Tensor-parallel over heads: 2 heads per core. Full inputs in, full output out.
Per core: qkv+g projections (fp32r matmuls; q,k,g in [chan,seq] layout, v in
[seq,chan]), per-head RMSNorm + neox RoPE (half-swap via permutation matmul),
chunked linear attention with decayed kv state in SBUF, group RMSNorm + gate,
AllToAll to sequence-shard y, then the dense projection for the core's
1024-row output shard.
"""
import math

import numpy as np

S = 8192
HID = 2048
H = 16
D = 128
BLK = 256
GROUPS = 8
EPS = 1e-5
ROPE_THETA = 600000.0
SCALE = D ** -0.5
N_CORES = 8
HPC = H // N_CORES          # heads per core = 2
CPC = HPC * D               # channels per core = 256
KT = 17                     # contraction tiles (2048 hid + bias row, padded)
KPAD = KT * 128             # 2176
SEQ_G = 512                 # seq per projection group
NG = S // SEQ_G             # 16 groups
SB = S // N_CORES           # seq block per core after AllToAll = 1024
CPG = SEQ_G // BLK          # chunks per group = 2

_cache = {}


def _build_slopes():
    start = 2.0 ** (-(2.0 ** (-(math.log2(H) - 3.0))))
    slopes = np.array([start * start ** i for i in range(H)], dtype=np.float32)
    return slopes * np.float32(1.0 - 0.0 / (20 - 1) + 1e-5)


def _build_program():
    import concourse.bacc as bacc
    import concourse.tile as tile
    import concourse.mybir as mybir
    from contextlib import ExitStack

    dt = mybir.dt
    AF = mybir.ActivationFunctionType
    OP = mybir.AluOpType

    nc = bacc.Bacc("TRN2", target_bir_lowering=False, debug=False,
                   num_devices=N_CORES)

    def din(name, shape, dtype=dt.float32):
        return nc.dram_tensor(name, shape, dtype, kind="ExternalInput").ap()

    hsT = din("hsT", [KPAD, S], dt.float32r)
    wB = din("wB", [KPAD, 768], dt.float32r)       # cols: q(256) k(256) g(256)
    wv = din("wv", [KPAD, 256], dt.float32r)
    dwT = din("dwT", [HID, HID], dt.float32r)
    cosf = din("cosf", [128, S])                    # [cos; cos]
    sinf = din("sinf", [128, S])                    # [-sin; sin]
    qdec_d = din("qdec", [128, HPC, BLK])
    kdec_d = din("kdec", [128, HPC, BLK])
    diag_d = din("diagT", [128, HPC, 2, BLK])
    qnw_d = din("qnw", [128, 1])
    knw_d = din("knw", [128, 1])
    gnw_d = din("gnw", [128, HPC])
    blk_d = din("blkdec", [128, HPC])
    ones128_d = din("ones128", [128, 1], dt.float32r)
    onesr_d = din("onesr", [1, 128], dt.float32r)   # bcast lhsT, value 1
    scaler_d = din("scaler", [1, 128], dt.float32r)  # bcast lhsT, value SCALE
    idm_d = din("idm", [128, 128], dt.float32r)
    idsw_d = din("idsw", [128, 128], dt.float32r)   # half-swap permutation
    zkv_d = din("zkv", [128, 128], dt.float32r)     # zeros for kv init

    out_d = nc.dram_tensor("out", [SB, HID], dt.float32,
                           kind="ExternalOutput").ap()

    with tile.TileContext(nc) as tc:
        ctx = ExitStack()
        main = ExitStack()
        consts = main.enter_context(tc.tile_pool(name="consts", bufs=1))
        wpool = main.enter_context(tc.tile_pool(name="wpool", bufs=1))
        dramp = ctx.enter_context(tc.tile_pool(name="dramp", bufs=1, space="DRAM"))
        y_send = dramp.tile([2, N_CORES, CPC, SB // 2], dt.float32r,
                            name="y_send")
        y_recv = dramp.tile([2, N_CORES, CPC, SB // 2], dt.float32r,
                            name="y_recv")

        def cload(name, ap_src, shape, dtype=dt.float32):
            t = consts.tile(shape, dtype, name=name, tag=name)
            nc.sync.dma_start(out=t[:], in_=ap_src)
            return t

        qdec = cload("qdec_s", qdec_d[:], [128, HPC, BLK])
        kdec = cload("kdec_s", kdec_d[:], [128, HPC, BLK])
        diag = cload("diag_s", diag_d[:], [128, HPC, 2, BLK])
        qnw = cload("qnw_s", qnw_d[:], [128, 1])
        knw = cload("knw_s", knw_d[:], [128, 1])
        gnw = cload("gnw_s", gnw_d[:], [128, HPC])
        blkd = cload("blkd_s", blk_d[:], [128, HPC])
        ones128 = cload("ones128_s", ones128_d[:], [128, 1], dt.float32r)
        onesr = cload("onesr_s", onesr_d[:], [1, 128], dt.float32r)
        scaler = cload("scaler_s", scaler_d[:], [1, 128], dt.float32r)
        idm = cload("idm_s", idm_d[:], [128, 128], dt.float32r)
        idsw = cload("idsw_s", idsw_d[:], [128, 128], dt.float32r)
        epsb = consts.tile([1, 1], dt.float32, name="epsb", tag="epsb")
        nc.vector.memset(epsb[:], EPS)

        wB_sb = wpool.tile([128, KT, 768], dt.float32r, name="wB_sb")
        nc.sync.dma_start(
            out=wB_sb[:], in_=wB.rearrange("(t p) c -> p t c", p=128))
        wv_sb = wpool.tile([128, KT, 256], dt.float32r, name="wv_sb")
        nc.sync.dma_start(
            out=wv_sb[:], in_=wv.rearrange("(t p) c -> p t c", p=128))

        hkp = main.enter_context(tc.tile_pool(name="hkp", bufs=17))
        tabp = main.enter_context(tc.tile_pool(name="tabp", bufs=1))
        evp = main.enter_context(tc.tile_pool(name="evp", bufs=2))
        xrp = main.enter_context(tc.tile_pool(name="xrp", bufs=1))
        natp = main.enter_context(tc.tile_pool(name="natp", bufs=1))
        attp = main.enter_context(tc.tile_pool(name="attp", bufs=2))
        kvp = main.enter_context(tc.tile_pool(name="kvp", bufs=1))
        yp = main.enter_context(tc.tile_pool(name="yp", bufs=2))
        psp = main.enter_context(tc.tile_pool(name="psp", bufs=3, space="PSUM"))
        pse = main.enter_context(tc.tile_pool(name="pse", bufs=2, space="PSUM"))
        psa = main.enter_context(tc.tile_pool(name="psa", bufs=3, space="PSUM"))

        # persistent kv state, ping-pong per head
        kv_sb = [[kvp.tile([128, 128], dt.float32r, name=f"kv{h}_{i}",
                           tag=f"kv{h}_{i}") for i in range(2)]
                 for h in range(HPC)]
        for h in range(HPC):
            nc.sync.dma_start(out=kv_sb[h][0][:], in_=zkv_d[:])

        for g in range(NG):
            s0 = g * SEQ_G
            hk = []
            for t in range(KT):
                hkt = hkp.tile([128, SEQ_G], dt.float32r,
                               name=f"hk{g}_{t}", tag="hk")
                nc.sync.dma_start(out=hkt[:],
                                  in_=hsT[t * 128:(t + 1) * 128, s0:s0 + SEQ_G])
                hk.append(hkt)
            cos_g = tabp.tile([128, SEQ_G], dt.float32, name=f"cos{g}", tag="cos")
            nc.sync.dma_start(out=cos_g[:], in_=cosf[:, s0:s0 + SEQ_G])
            sin_g = tabp.tile([128, SEQ_G], dt.float32, name=f"sin{g}", tag="sin")
            nc.sync.dma_start(out=sin_g[:], in_=sinf[:, s0:s0 + SEQ_G])

            # ---- projection accumulations (all matmuls first) --------------
            accs = []
            for ci in range(6):  # 0,1=q  2,3=k  4,5=g
                acc = psp.tile([128, SEQ_G], dt.float32,
                               name=f"acc{g}_{ci}", tag="ps")
                for t in range(KT):
                    nc.tensor.matmul(acc[:], wB_sb[:, t, ci * 128:(ci + 1) * 128],
                                     hk[t][:], start=(t == 0), stop=(t == KT - 1))
                accs.append(acc)
                if ci < 4:
                    # free the accumulator early via ACT copy
                    xb = evp.tile([128, SEQ_G], dt.float32,
                                  name=f"xb{g}_{ci}", tag="xb", bufs=3)
                    nc.scalar.activation(xb[:], acc[:], AF.Copy)
                    accs[ci] = xb
                else:
                    # g gate: sigmoid via exp (frees acc)
                    eg = evp.tile([128, SEQ_G], dt.float32,
                                  name=f"eg{g}_{ci}", tag="eg")
                    nc.scalar.activation(eg[:], acc[:], AF.Exp, scale=-1.0)
                    accs[ci] = eg
            v_accs = []
            for s2 in range(2):
                accv = psp.tile([128, SEQ_G], dt.float32,
                                name=f"accv{g}_{s2}", tag="ps")
                for half in range(2):
                    st = s2 * 2 + half
                    for t in range(KT):
                        nc.tensor.matmul(
                            accv[:, half * 256:(half + 1) * 256],
                            hk[t][:, st * 128:(st + 1) * 128],
                            wv_sb[:, t, :],
                            start=(t == 0), stop=(t == KT - 1))
                v_accs.append(accv)

            # ---- v eviction (DVE, natural layout) --------------------------
            v_nat = []
            for s2 in range(2):
                for half in range(2):
                    st = s2 * 2 + half
                    vn = natp.tile([128, 256], dt.float32r,
                                   name=f"vn{g}_{st}", tag=f"vn{st}")
                    nc.vector.tensor_copy(
                        vn[:], v_accs[s2][:, half * 256:(half + 1) * 256])
                    v_nat.append(vn)

            # ---- q/k norm + rope, g gate -----------------------------------
            qr_t, kr_t, ktil_t, sig_t = [], [], [], []
            for ci in range(6):
                if ci < 4:
                    is_q = ci < 2
                    xb = accs[ci]
                    sq = evp.tile([128, SEQ_G], dt.float32r,
                                  name=f"sq{g}_{ci}", tag="sq")
                    nc.vector.tensor_tensor(out=sq[:], in0=xb[:], in1=xb[:],
                                            op=OP.mult)
                    ssq = pse.tile([1, SEQ_G], dt.float32,
                                   name=f"ssq{g}_{ci}", tag="pse")
                    nc.tensor.matmul(ssq[:], ones128[:], sq[:],
                                     start=True, stop=True)
                    lnt = evp.tile([1, SEQ_G], dt.float32,
                                   name=f"ln{g}_{ci}", tag="ln")
                    nc.scalar.activation(lnt[:], ssq[:], AF.Ln,
                                         bias=epsb[:], scale=1.0 / D)
                    rstd = evp.tile([1, SEQ_G], dt.float32r,
                                    name=f"rstd{g}_{ci}", tag="rstd")
                    nc.scalar.activation(rstd[:], lnt[:], AF.Exp, scale=-0.5)
                    bc = pse.tile([128, SEQ_G], dt.float32,
                                  name=f"bc{g}_{ci}", tag="pse")
                    nc.tensor.matmul(bc[:], scaler[:] if is_q else onesr[:],
                                     rstd[:], start=True, stop=True)
                    bcs = evp.tile([128, SEQ_G], dt.float32,
                                   name=f"bcs{g}_{ci}", tag="bcs")
                    nc.vector.tensor_copy(bcs[:], bc[:])
                    xn = evp.tile([128, SEQ_G], dt.float32r,
                                  name=f"xn{g}_{ci}", tag="xn")
                    nc.vector.scalar_tensor_tensor(
                        out=xn[:], in0=xb[:], scalar=qnw[:] if is_q else knw[:],
                        in1=bcs[:], op0=OP.mult, op1=OP.mult)
                    # rope: xr = xn*cos + swap(xn)*sin_signed
                    m1 = evp.tile([128, SEQ_G], dt.float32,
                                  name=f"m1{g}_{ci}", tag="m1", bufs=1)
                    nc.vector.tensor_tensor(out=m1[:], in0=xn[:], in1=cos_g[:],
                                            op=OP.mult)
                    swp = pse.tile([128, SEQ_G], dt.float32,
                                   name=f"swp{g}_{ci}", tag="pse")
                    nc.tensor.matmul(swp[:], idsw[:], xn[:],
                                     start=True, stop=True)
                    m2 = evp.tile([128, SEQ_G], dt.float32,
                                  name=f"m2{g}_{ci}", tag="m2", bufs=1)
                    nc.vector.tensor_tensor(out=m2[:], in0=swp[:], in1=sin_g[:],
                                            op=OP.mult)
                    xr = xrp.tile([128, SEQ_G], dt.float32r,
                                  name=f"xr{g}_{ci}", tag=f"xr{ci}")
                    nc.vector.tensor_tensor(out=xr[:], in0=m1[:], in1=m2[:],
                                            op=OP.add)
                    if is_q:
                        qr_t.append(xr)
                    else:
                        kr_t.append(xr)
                else:
                    eg = accs[ci]
                    nc.vector.tensor_scalar_add(eg[:], eg[:], 1.0)
                    sig = xrp.tile([128, SEQ_G], dt.float32,
                                   name=f"sig{g}_{ci}", tag=f"sig{ci}")
                    nc.vector.reciprocal(sig[:], eg[:])
                    sig_t.append(sig)

            # decayed k for the kv update, per chunk
            for h in range(HPC):
                ktil = xrp.tile([128, SEQ_G], dt.float32r,
                                name=f"ktil{g}_{h}", tag=f"ktil{h}")
                for cc in range(CPG):
                    nc.vector.tensor_tensor(
                        out=ktil[:, cc * BLK:(cc + 1) * BLK],
                        in0=kr_t[h][:, cc * BLK:(cc + 1) * BLK],
                        in1=kdec[:, h, :], op=OP.mult)
                ktil_t.append(ktil)

            # ---- k natural (transpose of decayed k) ------------------------
            knat = [[None] * CPG for _ in range(HPC)]
            for h in range(HPC):
                for cc in range(CPG):
                    kn_list = []
                    for j in range(2):
                        tp = pse.tile([128, 128], dt.float32r,
                                      name=f"tp{g}_{h}_{cc}_{j}", tag="pse")
                        nc.tensor.transpose(
                            tp[:],
                            ktil_t[h][:, cc * BLK + j * 128:cc * BLK + (j + 1) * 128],
                            idm[:])
                        kn = natp.tile([128, 128], dt.float32r,
                                       name=f"kn{g}_{h}_{cc}_{j}", tag="kn",
                                       bufs=8)
                        nc.vector.tensor_copy(kn[:], tp[:])
                        kn_list.append(kn)
                    knat[h][cc] = kn_list

            # ---- attention chunks ------------------------------------------
            for cc in range(CPG):
                ch = g * CPG + cc
                b = ch // (SB // BLK)
                half = (ch % (SB // BLK)) // 2
                off = (ch % 2) * BLK
                o_ps = []
                for h in range(HPC):
                    qr = qr_t[h][:, cc * BLK:(cc + 1) * BLK]
                    kv_cur = kv_sb[h][ch % 2]
                    kv_nxt = kv_sb[h][(ch + 1) % 2]
                    # kq[j, i] masked
                    kq = psa.tile([128, SEQ_G], dt.float32,
                                  name=f"kq{ch}_{h}", tag="psa")
                    kqd = []
                    for j in range(2):
                        nc.tensor.matmul(
                            kq[:, j * BLK:(j + 1) * BLK],
                            kr_t[h][:, cc * BLK + j * 128:cc * BLK + (j + 1) * 128],
                            qr, start=True, stop=True)
                        kqj = attp.tile([128, BLK], dt.float32r,
                                        name=f"kqd{ch}_{h}_{j}", tag="kqd",
                                        bufs=3)
                        nc.vector.tensor_tensor(
                            out=kqj[:], in0=kq[:, j * BLK:(j + 1) * BLK],
                            in1=diag[:, h, j, :], op=OP.mult)
                        kqd.append(kqj)
                    # q with decay
                    qt = attp.tile([128, BLK], dt.float32r,
                                   name=f"qt{ch}_{h}", tag="qt")
                    nc.vector.tensor_tensor(out=qt[:], in0=qr,
                                            in1=qdec[:, h, :], op=OP.mult)
                    # outT = v0.T@kqd0 + v1.T@kqd1 + kv.T@qt
                    ops = psa.tile([128, BLK], dt.float32,
                                   name=f"ops{ch}_{h}", tag="psa")
                    for j in range(2):
                        nc.tensor.matmul(
                            ops[:], v_nat[cc * 2 + j][:, h * 128:(h + 1) * 128],
                            kqd[j][:], start=(j == 0), stop=False)
                    nc.tensor.matmul(ops[:], kv_cur[:], qt[:],
                                     start=False, stop=True)
                    o_ps.append(ops)
                    # kv update
                    kvp_ps = psa.tile([128, 128], dt.float32,
                                      name=f"kvp{ch}_{h}", tag="psa")
                    for j in range(2):
                        nc.tensor.matmul(
                            kvp_ps[:], knat[h][cc][j][:],
                            v_nat[cc * 2 + j][:, h * 128:(h + 1) * 128],
                            start=(j == 0), stop=(j == 1))
                    nc.vector.scalar_tensor_tensor(
                        out=kv_nxt[:], in0=kv_cur[:], scalar=blkd[:, h:h + 1],
                        in1=kvp_ps[:], op0=OP.mult, op1=OP.add)

                # group rmsnorm over both heads + gate
                sqs = []
                for h in range(HPC):
                    sqh = attp.tile([128, BLK], dt.float32r,
                                    name=f"gsq{ch}_{h}", tag="gsq", bufs=2)
                    nc.scalar.activation(sqh[:], o_ps[h][:], AF.Square)
                    sqs.append(sqh)
                gssq = psa.tile([1, BLK], dt.float32, name=f"gssq{ch}", tag="psa")
                for h in range(HPC):
                    nc.tensor.matmul(gssq[:], ones128[:], sqs[h][:],
                                     start=(h == 0), stop=(h == HPC - 1))
                glt = attp.tile([1, BLK], dt.float32, name=f"glt{ch}", tag="glt")
                nc.scalar.activation(glt[:], gssq[:], AF.Ln,
                                     bias=epsb[:], scale=1.0 / CPC)
                grstd = attp.tile([1, BLK], dt.float32r,
                                  name=f"grstd{ch}", tag="grstd")
                nc.scalar.activation(grstd[:], glt[:], AF.Exp, scale=-0.5)
                gbc = psa.tile([128, BLK], dt.float32, name=f"gbc{ch}", tag="psa")
                nc.tensor.matmul(gbc[:], onesr[:], grstd[:],
                                 start=True, stop=True)
                gbcs = attp.tile([128, BLK], dt.float32,
                                 name=f"gbcs{ch}", tag="gbcs")
                nc.vector.tensor_copy(gbcs[:], gbc[:])
                for h in range(HPC):
                    y1 = yp.tile([128, BLK], dt.float32,
                                 name=f"y1{ch}_{h}", tag="y1")
                    nc.vector.scalar_tensor_tensor(
                        out=y1[:], in0=o_ps[h][:], scalar=gnw[:, h:h + 1],
                        in1=gbcs[:], op0=OP.mult, op1=OP.mult)
                    y2 = yp.tile([128, BLK], dt.float32r,
                                 name=f"y2{ch}_{h}", tag="y2", bufs=3)
                    nc.vector.tensor_tensor(
                        out=y2[:], in0=y1[:],
                        in1=sig_t[h][:, cc * BLK:(cc + 1) * BLK], op=OP.mult)
                    nc.sync.dma_start(
                        out=y_send[half, b, h * 128:(h + 1) * 128,
                                   off:off + BLK],
                        in_=y2[:])

        main.close()

        # ---- all-to-all (two halves; first fires a group earlier) ----------
        for half in range(2):
            nc.gpsimd.collective_compute(
                "AllToAll", mybir.AluOpType.bypass,
                replica_groups=[list(range(N_CORES))],
                ins=[y_send[half].opt()],
                outs=[y_recv[half].opt()],
            )

        # ---- dense ---------------------------------------------------------
        dctx = ExitStack()
        dwp = dctx.enter_context(tc.tile_pool(name="dwp", bufs=2))
        dyp = dctx.enter_context(tc.tile_pool(name="dyp", bufs=6))
        dop = dctx.enter_context(tc.tile_pool(name="dop", bufs=4))
        dps = dctx.enter_context(tc.tile_pool(name="dps", bufs=1, space="PSUM"))
        for hh in range(2):
            dwt = dwp.tile([128, 16, 1024], dt.float32r,
                           name=f"dwt{hh}", tag="dw")
            nc.sync.dma_start(
                out=dwt[:],
                in_=dwT[:, hh * 1024:(hh + 1) * 1024]
                    .rearrange("(t p) c -> p t c", p=128))
            for half in range(2):
                accs = [[dps.tile([128, 512], dt.float32,
                                  name=f"dacc{hh}_{half}_{st}_{h2}",
                                  tag=f"dps{st}_{h2}")
                         for h2 in range(2)] for st in range(4)]
                for ct in range(16):
                    yt = dyp.tile([128, 512], dt.float32r,
                                  name=f"yt{hh}_{half}_{ct}", tag="yt")
                    nc.sync.dma_start(
                        out=yt[:],
                        in_=y_recv[half, ct // 2,
                                   (ct % 2) * 128:(ct % 2) * 128 + 128, :])
                    for st in range(4):
                        for h2 in range(2):
                            nc.tensor.matmul(
                                accs[st][h2][:],
                                yt[:, st * 128:(st + 1) * 128],
                                dwt[:, ct, h2 * 512:(h2 + 1) * 512],
                                start=(ct == 0), stop=(ct == 15))
                for st in range(4):
                    for h2 in range(2):
                        ot = dop.tile([128, 512], dt.float32,
                                      name=f"ot{hh}_{half}_{st}_{h2}", tag="ot")
                        nc.scalar.activation(ot[:], accs[st][h2][:], AF.Copy)
                        srow = half * 512 + st * 128
                        nc.sync.dma_start(
                            out=out_d[srow:srow + 128,
                                      hh * 1024 + h2 * 512:hh * 1024 + (h2 + 1) * 512],
                            in_=ot[:])
        dctx.close()
        ctx.close()

    nc.compile()
    return nc


def _stage(hidden_states, positions, qkv_w, qkv_b, q_norm_w, k_norm_w,
           g_w, g_norm_w, dense_w):
    f32 = np.float32
    hidden_states = np.asarray(hidden_states, dtype=f32)
    positions = np.asarray(positions)
    qkv_w = np.asarray(qkv_w, dtype=f32)
    qkv_b = np.asarray(qkv_b, dtype=f32)
    q_norm_w = np.asarray(q_norm_w, dtype=f32)
    k_norm_w = np.asarray(k_norm_w, dtype=f32)
    g_w = np.asarray(g_w, dtype=f32)
    g_norm_w = np.asarray(g_norm_w, dtype=f32)
    dense_w = np.asarray(dense_w, dtype=f32)
    slopes = _build_slopes()

    hsT = np.zeros((KPAD, S), dtype=f32)
    hsT[0:HID] = hidden_states.T
    hsT[HID] = 1.0

    inv_freq = 1.0 / (ROPE_THETA ** (np.arange(0, D, 2, dtype=f32) / D))
    freqs = positions.astype(f32)[:, None] * inv_freq[None, :]  # [S, 64]
    cos = np.cos(freqs).T.astype(f32)     # [64, S]
    sin = np.sin(freqs).T.astype(f32)
    cosf = np.ascontiguousarray(np.concatenate([cos, cos], axis=0))
    sinf = np.ascontiguousarray(np.concatenate([-sin, sin], axis=0))

    idx = np.arange(BLK, dtype=f32)
    dwT = np.ascontiguousarray(dense_w.T).astype(f32)
    ones128 = np.ones((128, 1), dtype=f32)
    onesr = np.ones((1, 128), dtype=f32)
    scaler = np.full((1, 128), SCALE, dtype=f32)
    idm = np.eye(128, dtype=f32)
    idsw = np.zeros((128, 128), dtype=f32)
    for m in range(128):
        idsw[(m + 64) % 128, m] = 1.0
    qnw = q_norm_w.reshape(128, 1).copy()
    knw = k_norm_w.reshape(128, 1).copy()

    in_maps = []
    for j in range(N_CORES):
        heads = [j * HPC + h for h in range(HPC)]
        c0 = j * CPC
        wBm = np.zeros((KPAD, 768), dtype=f32)
        wBm[0:HID, 0:256] = qkv_w[c0:c0 + CPC, :].T
        wBm[0:HID, 256:512] = qkv_w[HID + c0:HID + c0 + CPC, :].T
        wBm[0:HID, 512:768] = g_w[c0:c0 + CPC, :].T
        wBm[HID, 0:256] = qkv_b[c0:c0 + CPC]
        wBm[HID, 256:512] = qkv_b[HID + c0:HID + c0 + CPC]
        wvm = np.zeros((KPAD, 256), dtype=f32)
        wvm[0:HID] = qkv_w[2 * HID + c0:2 * HID + c0 + CPC, :].T
        wvm[HID] = qkv_b[2 * HID + c0:2 * HID + c0 + CPC]

        sl = slopes[heads]  # [HPC]
        qdec = np.exp(-sl[:, None] * (idx + 1.0)[None, :]).astype(f32)
        qdec = np.ascontiguousarray(
            np.broadcast_to(qdec[None, :, :], (128, HPC, BLK)))
        kd = np.exp(-sl[:, None] * (BLK - 1.0 - idx)[None, :]).astype(f32)
        kdecm = np.ascontiguousarray(
            np.broadcast_to(kd[None, :, :], (128, HPC, BLK)))
        dif = idx[:, None] - idx[None, :]           # [i, j]
        diagT = np.zeros((128, HPC, 2, BLK), dtype=f32)
        for hh in range(HPC):
            dd = np.where(
                dif >= 0,
                np.exp(-sl[hh] * np.where(dif >= 0, dif, 0.0)),
                0.0).astype(f32)                    # [i, j]
            ddT = dd.T                               # [j, i]
            diagT[:, hh, 0, :] = ddT[0:128]
            diagT[:, hh, 1, :] = ddT[128:256]
        blkdec = np.ascontiguousarray(np.broadcast_to(
            np.exp(-sl * BLK).astype(f32)[None, :], (128, HPC)))
        gnwm = np.ascontiguousarray(g_norm_w[c0:c0 + CPC].reshape(HPC, 128).T)

        in_maps.append({
            "hsT": hsT, "wB": wBm, "wv": wvm, "dwT": dwT,
            "cosf": cosf, "sinf": sinf,
            "qdec": qdec, "kdec": kdecm, "diagT": diagT,
            "qnw": qnw, "knw": knw, "gnw": gnwm, "blkdec": blkdec,
            "ones128": ones128, "onesr": onesr, "scaler": scaler,
            "idm": idm, "idsw": idsw, "zkv": np.zeros((128, 128), dtype=f32),
        })
    return in_maps


def kernel(**inputs):
    from concourse.bass_utils import run_bass_kernel_spmd

    if "nc" not in _cache:
        _cache["nc"] = _build_program()
    nc = _cache["nc"]
    in_maps = _stage(**inputs)
    res = run_bass_kernel_spmd(nc, in_maps, list(range(N_CORES)))
    out = np.concatenate([res.results[j]["out"] for j in range(N_CORES)],
                         axis=0)
    return out.astype(np.float32)
